# revision 18
# baseline (speedup 1.0000x reference)
"""Trainium2 Bass kernel for AttnSum3d pooling.

Math (per batch):
    xm = input * mask[:, None]                      # [L, D]
    S  = xm @ xm.T                                  # [L, L], symmetric
    w  = softmax(S, axis=0)  (columns sum to 1 over rows)
    out[d]       = (1/L) * sum_m sum_l w[l, m] xm[l, d]
    attn_mean[m] = (1/L) * sum_l w[l, m] = 1/L      (constant!)

Kernel computes, per m-block of 128 columns (stored row-wise thanks to
symmetry: S_j[m, l] for m in block j):
    c[m]      = sqrt(nsq[m] * max_l nsq[l]) >= max_l S[l, m]   (Cauchy-Schwarz)
    P_j[m, l] = exp(S_j[m, l] - c[m])        (ACT, bias=-c, accum_out=colsum)
    v[m]      = 1 / colsum[m]
    r[l]     += sum_m v[m] * P_j[m, l]       (TensorE, lhsT=v)
then out = (1/L) * r @ xm via 16 accumulated [128,1]x[128,128] matmuls.

Data-parallel over batch: 16 batches -> 8 cores x 2 batches.
"""

import sys

for _p in ("/opt/trn_rl_repo",):
    if _p not in sys.path:
        sys.path.insert(0, _p)

import numpy as np

B, L, D = 16, 2048, 128
NCORES = 8
BPC = B // NCORES          # batches per core
NT = L // 128              # 16 tiles of 128 along L
TPB = L // 128             # elements of L per partition in the (p t) layout

_CACHE = {}


def _build_nc(stage=4, batches=BPC):
    import concourse.bacc as bacc
    import concourse.tile as tile
    from concourse import mybir
    from concourse.masks import make_identity

    f32 = mybir.dt.float32
    f32r = mybir.dt.float32r
    bf16 = mybir.dt.bfloat16
    AF = mybir.ActivationFunctionType
    ALU = mybir.AluOpType
    AX = mybir.AxisListType

    nc = bacc.Bacc("TRN2", target_bir_lowering=False, debug=False)

    x_d = nc.dram_tensor("input", [BPC, L, D], f32, kind="ExternalInput").ap()
    m_d = nc.dram_tensor("mask", [BPC, L], f32, kind="ExternalInput").ap()
    o_d = nc.dram_tensor("out", [BPC, D], f32, kind="ExternalOutput").ap()

    with tile.TileContext(nc) as tc:
        with (
            tc.tile_pool(name="consts", bufs=1) as consts,
            tc.tile_pool(name="xb", bufs=2) as xb,
            tc.tile_pool(name="pb", bufs=4) as pb,
            tc.tile_pool(name="small", bufs=4) as small,
            tc.tile_pool(name="psS", bufs=2, space="PSUM") as psS,
            tc.tile_pool(name="psR", bufs=1, space="PSUM") as psR,
        ):
            identity = consts.tile([128, 128], f32)
            make_identity(nc, identity)
            ones_row = consts.tile([1, 128], f32)
            nc.vector.memset(ones_row, 1.0)
            eps_col = consts.tile([128, 1], f32)
            nc.vector.memset(eps_col, 1e-30)
            zero_col = consts.tile([128, 1], f32)
            nc.vector.memset(zero_col, 0.0)

            for b in range(batches):
                # ---------------- load ----------------
                # partition p holds rows l = TPB*p + t  (16 contiguous rows
                # = 8KB per partition -> single fat DMA descriptor each)
                xp = xb.tile([128, TPB, D], f32, name=f"xp{b}", tag="xp")
                nc.sync.dma_start(
                    out=xp[:], in_=x_d[b].rearrange("(p t) d -> p t d", p=128)
                )
                mask_sb = xb.tile([128, TPB], f32, name=f"mask{b}", tag="mask")
                nc.sync.dma_start(
                    out=mask_sb[:], in_=m_d[b].rearrange("(p t) -> p t", p=128)
                )

                # ---------------- xm = x*mask, nsq = |xm|^2 ----------------
                xm = xb.tile([128, NT, D], f32, name=f"xm{b}", tag="xm")
                nsq = xb.tile([128, NT], f32, name=f"nsq{b}", tag="nsq")
                sq = xb.tile([128, D], f32, name=f"sq{b}", tag="sq")
                for t in range(NT):
                    nc.vector.tensor_scalar_mul(
                        xm[:, t, :], xp[:, t, :], mask_sb[:, t : t + 1]
                    )
                for t in range(NT):
                    nc.vector.tensor_mul(sq[:], xm[:, t, :], xm[:, t, :])
                    nc.vector.reduce_sum(nsq[:, t : t + 1], sq[:], AX.X)

                # ---------------- N2max = max_l nsq[l], broadcast ----------
                nmaxp = xb.tile([128, 1], f32, name=f"nmaxp{b}", tag="nmaxp")
                nc.vector.reduce_max(nmaxp[:], nsq[:], AX.X)
                tps = psS.tile([1, 128], f32, name=f"tpn{b}", tag="ps")
                nc.tensor.transpose(tps[:], nmaxp[:], identity[:])
                nmax_row = xb.tile([1, 128], f32, name=f"nmr{b}", tag="nmr")
                nc.vector.tensor_copy(nmax_row[:], tps[:])
                n2max = xb.tile([1, 1], f32, name=f"n2max{b}", tag="n2max")
                nc.vector.reduce_max(n2max[:], nmax_row[:], AX.X)
                bps = psR.tile([128, 1], f32, name=f"bps{b}", tag="r")
                nc.tensor.matmul(bps[:], ones_row[:], n2max[:], start=True, stop=True)
                n2b = xb.tile([128, 1], f32, name=f"n2b{b}", tag="n2b")
                nc.vector.tensor_copy(n2b[:], bps[:])

                # ---------------- c = sqrt(nsq*N2max) via ln/exp -----------
                # (keeps ACT on the natural_log_exp table set: no reload)
                lnu = xb.tile([128, NT], f32, name=f"lnu{b}", tag="lnu")
                nc.scalar.activation(
                    lnu[:], nsq[:], AF.Ln, bias=eps_col[:], scale=n2b[:, 0:1]
                )
                cpos = xb.tile([128, NT], f32, name=f"cpos{b}", tag="cpos")
                nc.scalar.activation(cpos[:], lnu[:], AF.Exp, bias=zero_col[:], scale=0.5)
                negc = xb.tile([128, NT], f32, name=f"negc{b}", tag="negc")
                nc.vector.tensor_scalar_mul(negc[:], cpos[:], -1.0)

                # ---------------- xmT via TensorE transpose ----------------
                # float32r: the DVE copy rounds f32 -> f32r so the PE can
                # stream it at 1 cyc/row (vs 4 for plain f32)
                xmT = xb.tile([128, L], f32r, name=f"xmT{b}", tag="xmT")
                for t in range(NT):
                    tp = psS.tile([128, 128], f32, name=f"tp{b}_{t}", tag="ps")
                    nc.tensor.transpose(tp[:], xm[:, t, :], identity[:])
                    nc.vector.tensor_copy(xmT[:, t * 128 : (t + 1) * 128], tp[:])

                if stage <= 1:
                    o_sb1 = xb.tile([1, D], f32, name=f"o_sb{b}", tag="o_sb")
                    nc.vector.memset(o_sb1[:], 0.5)
                    nc.vector.tensor_copy(o_sb1[0:1, 0:1], negc[0:1, 0:1])
                    nc.sync.dma_start(out=o_d[b : b + 1, :], in_=o_sb1[:])
                    continue

                # ---------------- main loop over m-blocks ------------------
                if stage >= 3:
                    r_ps = psR.tile([1, L], f32, name=f"r_ps{b}", tag="r")
                for jb in range(NT):
                    lhsT = xmT[:, jb * 128 : (jb + 1) * 128]
                    csum = small.tile([128, 2], f32, name=f"cs{b}_{jb}", tag="cs")
                    Ph = []
                    for h in range(2):
                        S_ps = psS.tile(
                            [128, 1024], f32, name=f"S{b}_{jb}_{h}", tag="ps"
                        )
                        for k in range(2):
                            nc.tensor.matmul(
                                S_ps[:, k * 512 : (k + 1) * 512],
                                lhsT,
                                xmT[
                                    :, h * 1024 + k * 512 : h * 1024 + (k + 1) * 512
                                ],
                                start=True,
                                stop=True,
                            )
                        P = pb.tile([128, 1024], bf16, name=f"P{b}_{jb}_{h}", tag="P")
                        nc.scalar.activation(
                            P[:],
                            S_ps[:],
                            AF.Exp,
                            bias=negc[:, jb : jb + 1],
                            scale=1.0,
                            accum_out=csum[:, h : h + 1],
                        )
                        Ph.append(P)

                    cst = small.tile([128, 1], f32, name=f"cst{b}_{jb}", tag="cst")
                    nc.vector.tensor_add(cst[:], csum[:, 0:1], csum[:, 1:2])
                    vj = small.tile([128, 1], f32, name=f"vj{b}_{jb}", tag="vj")
                    nc.vector.reciprocal(vj[:], cst[:])
                    vjb = small.tile([128, 1], bf16, name=f"vjb{b}_{jb}", tag="vjb")
                    nc.vector.tensor_copy(vjb[:], vj[:])

                    if stage >= 3:
                        for h in range(2):
                            for k in range(2):
                                nc.tensor.matmul(
                                    r_ps[
                                        0:1,
                                        h * 1024 + k * 512 : h * 1024 + (k + 1) * 512,
                                    ],
                                    vjb[:],
                                    Ph[h][:, k * 512 : (k + 1) * 512],
                                    start=(jb == 0),
                                    stop=(jb == NT - 1),
                                )

                if stage <= 2:
                    o_sb2 = xb.tile([1, D], f32, name=f"o_sb{b}", tag="o_sb")
                    nc.vector.memset(o_sb2[:], 0.5)
                    nc.vector.tensor_copy(o_sb2[0:1, 0:1], vjb[0:1, 0:1])
                    nc.sync.dma_start(out=o_d[b : b + 1, :], in_=o_sb2[:])
                    continue

                # ---------------- out = (1/L) * r @ xm ---------------------
                r_sb = xb.tile([1, L], f32, name=f"r_sb{b}", tag="r_sb")
                nc.vector.tensor_scalar_mul(r_sb[:], r_ps[:], 1.0 / L)
                if stage <= 3:
                    o_sb3 = xb.tile([1, D], f32, name=f"o_sb{b}", tag="o_sb")
                    nc.vector.tensor_copy(o_sb3[:], r_sb[0:1, 0:D])
                    nc.sync.dma_start(out=o_d[b : b + 1, :], in_=o_sb3[:])
                    continue
                rT = xb.tile([128, NT], f32, name=f"rT{b}", tag="rT")
                for i in range(NT):
                    tpr = psS.tile([128, 1], f32, name=f"tpr{b}_{i}", tag="ps")
                    nc.tensor.transpose(
                        tpr[:], r_sb[0:1, i * 128 : (i + 1) * 128], identity[0:1, 0:1]
                    )
                    nc.vector.tensor_copy(rT[:, i : i + 1], tpr[:])

                o_ps = psR.tile([1, D], f32, name=f"o_ps{b}", tag="r")
                for i in range(NT):
                    nc.tensor.matmul(
                        o_ps[:],
                        rT[:, i : i + 1],
                        xm[:, i, :],
                        start=(i == 0),
                        stop=(i == NT - 1),
                    )
                o_sb = xb.tile([1, D], f32, name=f"o_sb{b}", tag="o_sb")
                nc.vector.tensor_copy(o_sb[:], o_ps[:])
                nc.sync.dma_start(out=o_d[b : b + 1, :], in_=o_sb[:])

    nc.compile()
    return nc


def _get_nc():
    import os

    stage = int(os.environ.get("K_STAGE", "4"))
    batches = int(os.environ.get("K_BATCHES", str(BPC)))
    key = ("nc", stage, batches)
    if key not in _CACHE:
        _CACHE[key] = _build_nc(stage=stage, batches=batches)
    return _CACHE[key]


def _in_maps(inputs):
    x = np.ascontiguousarray(np.asarray(inputs["input"], dtype=np.float32))
    m = np.ascontiguousarray(np.asarray(inputs["mask"], dtype=np.float32))
    assert x.shape == (B, L, D) and m.shape == (B, L)
    return [
        {
            "input": np.ascontiguousarray(x[c * BPC : (c + 1) * BPC]),
            "mask": np.ascontiguousarray(m[c * BPC : (c + 1) * BPC]),
        }
        for c in range(NCORES)
    ]


def _enable_tracing():
    """Shim antenv.axon_hooks (absent in this container) so
    run_bass_kernel_spmd(trace=True) can capture NTFF profiles through
    the axon .so, and neutralize the S3 artifact upload."""
    if _CACHE.get("trace_shim"):
        return
    import types

    import antenv

    if not hasattr(antenv, "axon_hooks"):
        mod = types.ModuleType("antenv.axon_hooks")
        mod._hook = None

        def set_axon_ntff_profile_hook(h):
            mod._hook = h

        def get_axon_ntff_profile_hook():
            return mod._hook

        mod.set_axon_ntff_profile_hook = set_axon_ntff_profile_hook
        mod.get_axon_ntff_profile_hook = get_axon_ntff_profile_hook
        sys.modules["antenv.axon_hooks"] = mod
        antenv.axon_hooks = mod

    from antenv.axon_hooks import get_axon_ntff_profile_hook, set_axon_ntff_profile_hook

    if get_axon_ntff_profile_hook() is None:
        if "/root/.axon_site" not in sys.path:
            sys.path.insert(0, "/root/.axon_site")
        from trn_agent_boot.trn_boot import _ntff_profile_via_ctypes

        set_axon_ntff_profile_hook(
            _ntff_profile_via_ctypes("/opt/axon/libaxon_pjrt.so")
        )

    import concourse.bass_utils as bu

    bu.upload_artifacts = lambda tmpdir: f"local://{tmpdir}"
    _CACHE["trace_shim"] = True


def _run(inputs, trace=False, **kw):
    from concourse.bass_utils import run_bass_kernel_spmd

    if trace:
        _enable_tracing()
    nc = _get_nc()
    res = run_bass_kernel_spmd(
        nc, _in_maps(inputs), core_ids=list(range(NCORES)), trace=trace, **kw
    )
    outs = np.stack([res.results[c]["out"] for c in range(NCORES)])  # [8, BPC, D]
    out_full = outs.reshape(B, 1, D).astype(np.float32)
    attn_mean = np.full((B, L), 1.0 / L, dtype=np.float32)
    return (out_full, attn_mean), res


def kernel(**inputs):
    (out_full, attn_mean), _ = _run(inputs, trace=False)
    return (out_full, attn_mean)


# revision 23
# speedup vs baseline: 1.0180x; 1.0180x over previous
"""Trainium2 Bass kernel for AttnSum3d pooling.

Math (per batch):
    xm = input * mask[:, None]                      # [L, D]
    S  = xm @ xm.T                                  # [L, L], symmetric
    w  = softmax(S, axis=0)  (columns sum to 1 over rows)
    out[d]       = (1/L) * sum_m sum_l w[l, m] xm[l, d]
    attn_mean[m] = (1/L) * sum_l w[l, m] = 1/L      (constant!)

Kernel computes, per m-block of 128 columns (stored row-wise thanks to
symmetry: S_j[m, l] for m in block j):
    c[m]      = sqrt(nsq[m] * max_l nsq[l]) >= max_l S[l, m]   (Cauchy-Schwarz)
    P_j[m, l] = exp(S_j[m, l] - c[m])        (ACT, bias=-c, accum_out=colsum)
    v[m]      = 1 / colsum[m]
    r[l]     += sum_m v[m] * P_j[m, l]       (TensorE, lhsT=v)
then out = (1/L) * r @ xm via 16 accumulated [128,1]x[128,128] matmuls.

Data-parallel over batch: 16 batches -> 8 cores x 2 batches.
"""

import sys

for _p in ("/opt/trn_rl_repo",):
    if _p not in sys.path:
        sys.path.insert(0, _p)

import numpy as np

B, L, D = 16, 2048, 128
NCORES = 8
BPC = B // NCORES          # batches per core
NT = L // 128              # 16 tiles of 128 along L
TPB = L // 128             # elements of L per partition in the (p t) layout

_CACHE = {}


def _build_nc(stage=4, batches=BPC):
    import concourse.bacc as bacc
    import concourse.tile as tile
    from concourse import mybir
    from concourse.masks import make_identity

    f32 = mybir.dt.float32
    f32r = mybir.dt.float32r
    bf16 = mybir.dt.bfloat16
    AF = mybir.ActivationFunctionType
    ALU = mybir.AluOpType
    AX = mybir.AxisListType

    nc = bacc.Bacc("TRN2", target_bir_lowering=False, debug=False)

    x_d = nc.dram_tensor("input", [BPC, L, D], f32, kind="ExternalInput").ap()
    m_d = nc.dram_tensor("mask", [BPC, L], f32, kind="ExternalInput").ap()
    o_d = nc.dram_tensor("out", [BPC, D], f32, kind="ExternalOutput").ap()

    with tile.TileContext(nc) as tc:
        with (
            tc.tile_pool(name="consts", bufs=1) as consts,
            tc.tile_pool(name="xb", bufs=2) as xb,
            tc.tile_pool(name="pb", bufs=4) as pb,
            tc.tile_pool(name="small", bufs=4) as small,
            tc.tile_pool(name="psS", bufs=2, space="PSUM") as psS,
            tc.tile_pool(name="psR", bufs=1, space="PSUM") as psR,
        ):
            identity = consts.tile([128, 128], f32)
            make_identity(nc, identity)
            ones_row = consts.tile([1, 128], f32)
            nc.vector.memset(ones_row, 1.0)

            for b in range(batches):
                # ---------------- load ----------------
                # partition p holds rows l = TPB*p + t  (16 contiguous rows
                # = 8KB per partition -> single fat DMA descriptor each)
                xp = xb.tile([128, TPB, D], f32, name=f"xp{b}", tag="xp")
                xsrc = x_d[b].rearrange("(p t) d -> p t d", p=128)
                for c in range(8):  # split across DMA queues
                    nc.sync.dma_start(
                        out=xp[:, 2 * c : 2 * c + 2, :], in_=xsrc[:, 2 * c : 2 * c + 2, :]
                    )
                mask_sb = xb.tile([128, TPB], f32, name=f"mask{b}", tag="mask")
                nc.sync.dma_start(
                    out=mask_sb[:], in_=m_d[b].rearrange("(p t) -> p t", p=128)
                )

                # ---------------- xm = x*mask, nsq = |xm|^2 ----------------
                xm = xb.tile([128, NT, D], f32, name=f"xm{b}", tag="xm")
                nsq = xb.tile([128, NT], f32, name=f"nsq{b}", tag="nsq")
                sq = xb.tile([128, D], f32, name=f"sq{b}", tag="sq")
                for t in range(NT):
                    nc.vector.tensor_scalar_mul(
                        xm[:, t, :], xp[:, t, :], mask_sb[:, t : t + 1]
                    )
                for t in range(NT):
                    nc.vector.tensor_mul(sq[:], xm[:, t, :], xm[:, t, :])
                    nc.vector.reduce_sum(nsq[:, t : t + 1], sq[:], AX.X)

                # ---------------- N2max = max_l nsq[l], broadcast ----------
                nmaxp = xb.tile([128, 1], f32, name=f"nmaxp{b}", tag="nmaxp")
                nc.vector.reduce_max(nmaxp[:], nsq[:], AX.X)
                tps = psS.tile([1, 128], f32, name=f"tpn{b}", tag="ps")
                nc.tensor.transpose(tps[:], nmaxp[:], identity[:])
                nmax_row = xb.tile([1, 128], f32, name=f"nmr{b}", tag="nmr")
                nc.vector.tensor_copy(nmax_row[:], tps[:])
                n2max = xb.tile([1, 1], f32, name=f"n2max{b}", tag="n2max")
                nc.vector.reduce_max(n2max[:], nmax_row[:], AX.X)
                bps = psR.tile([128, 1], f32, name=f"bps{b}", tag="r")
                nc.tensor.matmul(bps[:], ones_row[:], n2max[:], start=True, stop=True)
                n2b = xb.tile([128, 1], f32, name=f"n2b{b}", tag="n2b")
                nc.vector.tensor_copy(n2b[:], bps[:])

                # ---------------- c ~ sqrt(nsq*N2max), all on DVE ----------
                # exponent-halving sqrt approximation (+/-3.5%), scaled by
                # 1.06 so c >= true column max; keeps ACT exp-table resident
                zt = xb.tile([128, NT], f32, name=f"zt{b}", tag="zt")
                nc.vector.tensor_scalar_mul(zt[:], nsq[:], n2b[:, 0:1])
                zi = zt[:].bitcast(mybir.dt.int32)
                nc.vector.tensor_scalar(
                    zi, zi, 1, None, op0=ALU.arith_shift_right
                )
                nc.vector.tensor_scalar(
                    zi, zi, 0x1FC00000, None, op0=ALU.add
                )
                negc = xb.tile([128, NT], f32, name=f"negc{b}", tag="negc")
                nc.vector.tensor_scalar_mul(negc[:], zt[:], -1.06)

                # ---------------- xmT (bf16) via DMA transpose -------------
                xm_bf = xb.tile([128, NT, D], bf16, name=f"xmbf{b}", tag="xmbf")
                nc.vector.tensor_copy(xm_bf[:], xm[:])
                xmT = xb.tile([128, L], bf16, name=f"xmT{b}", tag="xmT")
                for t in range(NT):
                    nc.sync.dma_start_transpose(
                        out=xmT[:, t * 128 : (t + 1) * 128], in_=xm_bf[:, t, :]
                    )

                if stage <= 1:
                    o_sb1 = xb.tile([1, D], f32, name=f"o_sb{b}", tag="o_sb")
                    nc.vector.memset(o_sb1[:], 0.5)
                    nc.vector.tensor_copy(o_sb1[0:1, 0:1], negc[0:1, 0:1])
                    nc.sync.dma_start(out=o_d[b : b + 1, :], in_=o_sb1[:])
                    continue

                # ---------------- main loop over m-blocks ------------------
                if stage >= 3:
                    r_ps = psR.tile([1, L], f32, name=f"r_ps{b}", tag="r")
                for jb in range(NT):
                    lhsT = xmT[:, jb * 128 : (jb + 1) * 128]
                    csum = small.tile([128, 2], f32, name=f"cs{b}_{jb}", tag="cs")
                    Ph = []
                    for h in range(2):
                        S_ps = psS.tile(
                            [128, 1024], f32, name=f"S{b}_{jb}_{h}", tag="ps"
                        )
                        for k in range(2):
                            nc.tensor.matmul(
                                S_ps[:, k * 512 : (k + 1) * 512],
                                lhsT,
                                xmT[
                                    :, h * 1024 + k * 512 : h * 1024 + (k + 1) * 512
                                ],
                                start=True,
                                stop=True,
                            )
                        P = pb.tile([128, 1024], bf16, name=f"P{b}_{jb}_{h}", tag="P")
                        nc.scalar.activation(
                            P[:],
                            S_ps[:],
                            AF.Exp,
                            bias=negc[:, jb : jb + 1],
                            scale=1.0,
                            accum_out=csum[:, h : h + 1],
                        )
                        Ph.append(P)

                    cst = small.tile([128, 1], f32, name=f"cst{b}_{jb}", tag="cst")
                    nc.vector.tensor_add(cst[:], csum[:, 0:1], csum[:, 1:2])
                    vj = small.tile([128, 1], f32, name=f"vj{b}_{jb}", tag="vj")
                    nc.vector.reciprocal(vj[:], cst[:])
                    vjb = small.tile([128, 1], bf16, name=f"vjb{b}_{jb}", tag="vjb")
                    nc.vector.tensor_scalar_mul(vjb[:], vj[:], 1.0 / L)

                    if stage >= 3:
                        for h in range(2):
                            for k in range(2):
                                nc.tensor.matmul(
                                    r_ps[
                                        0:1,
                                        h * 1024 + k * 512 : h * 1024 + (k + 1) * 512,
                                    ],
                                    vjb[:],
                                    Ph[h][:, k * 512 : (k + 1) * 512],
                                    start=(jb == 0),
                                    stop=(jb == NT - 1),
                                )

                if stage <= 2:
                    o_sb2 = xb.tile([1, D], f32, name=f"o_sb{b}", tag="o_sb")
                    nc.vector.memset(o_sb2[:], 0.5)
                    nc.vector.tensor_copy(o_sb2[0:1, 0:1], vjb[0:1, 0:1])
                    nc.sync.dma_start(out=o_d[b : b + 1, :], in_=o_sb2[:])
                    continue

                # ---------------- out = (1/L) * r @ xm ---------------------
                r_sb = xb.tile([1, L], f32, name=f"r_sb{b}", tag="r_sb")
                nc.vector.tensor_copy(r_sb[:], r_ps[:])
                if stage <= 3:
                    o_sb3 = xb.tile([1, D], f32, name=f"o_sb{b}", tag="o_sb")
                    nc.vector.tensor_copy(o_sb3[:], r_sb[0:1, 0:D])
                    nc.sync.dma_start(out=o_d[b : b + 1, :], in_=o_sb3[:])
                    continue
                rT = xb.tile([128, NT], f32, name=f"rT{b}", tag="rT")
                for i in range(NT):
                    tpr = psS.tile([128, 1], f32, name=f"tpr{b}_{i}", tag="ps")
                    nc.tensor.transpose(
                        tpr[:], r_sb[0:1, i * 128 : (i + 1) * 128], identity[0:1, 0:1]
                    )
                    nc.vector.tensor_copy(rT[:, i : i + 1], tpr[:])

                o_ps = psR.tile([1, D], f32, name=f"o_ps{b}", tag="r")
                for i in range(NT):
                    nc.tensor.matmul(
                        o_ps[:],
                        rT[:, i : i + 1],
                        xm[:, i, :],
                        start=(i == 0),
                        stop=(i == NT - 1),
                    )
                o_sb = xb.tile([1, D], f32, name=f"o_sb{b}", tag="o_sb")
                nc.vector.tensor_copy(o_sb[:], o_ps[:])
                nc.sync.dma_start(out=o_d[b : b + 1, :], in_=o_sb[:])

    nc.compile()
    return nc


def _get_nc():
    import os

    stage = int(os.environ.get("K_STAGE", "4"))
    batches = int(os.environ.get("K_BATCHES", str(BPC)))
    key = ("nc", stage, batches)
    if key not in _CACHE:
        _CACHE[key] = _build_nc(stage=stage, batches=batches)
    return _CACHE[key]


def _in_maps(inputs):
    x = np.ascontiguousarray(np.asarray(inputs["input"], dtype=np.float32))
    m = np.ascontiguousarray(np.asarray(inputs["mask"], dtype=np.float32))
    assert x.shape == (B, L, D) and m.shape == (B, L)
    return [
        {
            "input": np.ascontiguousarray(x[c * BPC : (c + 1) * BPC]),
            "mask": np.ascontiguousarray(m[c * BPC : (c + 1) * BPC]),
        }
        for c in range(NCORES)
    ]


def _enable_tracing():
    """Shim antenv.axon_hooks (absent in this container) so
    run_bass_kernel_spmd(trace=True) can capture NTFF profiles through
    the axon .so, and neutralize the S3 artifact upload."""
    if _CACHE.get("trace_shim"):
        return
    import types

    import antenv

    if not hasattr(antenv, "axon_hooks"):
        mod = types.ModuleType("antenv.axon_hooks")
        mod._hook = None

        def set_axon_ntff_profile_hook(h):
            mod._hook = h

        def get_axon_ntff_profile_hook():
            return mod._hook

        mod.set_axon_ntff_profile_hook = set_axon_ntff_profile_hook
        mod.get_axon_ntff_profile_hook = get_axon_ntff_profile_hook
        sys.modules["antenv.axon_hooks"] = mod
        antenv.axon_hooks = mod

    from antenv.axon_hooks import get_axon_ntff_profile_hook, set_axon_ntff_profile_hook

    if get_axon_ntff_profile_hook() is None:
        if "/root/.axon_site" not in sys.path:
            sys.path.insert(0, "/root/.axon_site")
        from trn_agent_boot.trn_boot import _ntff_profile_via_ctypes

        set_axon_ntff_profile_hook(
            _ntff_profile_via_ctypes("/opt/axon/libaxon_pjrt.so")
        )

    import concourse.bass_utils as bu

    bu.upload_artifacts = lambda tmpdir: f"local://{tmpdir}"
    _CACHE["trace_shim"] = True


def _run(inputs, trace=False, **kw):
    from concourse.bass_utils import run_bass_kernel_spmd

    if trace:
        _enable_tracing()
    nc = _get_nc()
    res = run_bass_kernel_spmd(
        nc, _in_maps(inputs), core_ids=list(range(NCORES)), trace=trace, **kw
    )
    outs = np.stack([res.results[c]["out"] for c in range(NCORES)])  # [8, BPC, D]
    out_full = outs.reshape(B, 1, D).astype(np.float32)
    attn_mean = np.full((B, L), 1.0 / L, dtype=np.float32)
    return (out_full, attn_mean), res


def kernel(**inputs):
    (out_full, attn_mean), _ = _run(inputs, trace=False)
    return (out_full, attn_mean)
